# revision 1
# baseline (speedup 1.0000x reference)
"""Multi-head self-attention Trainium2 kernel (Bass/Tile), v6.

Problem: x:(8,256,32,32), 8 heads, head_dim=32, N=H*W=1024.
Sharding: data-parallel over batch B=8 -> one batch element per NeuronCore.

Per-core math (b fixed, X = x[b] as (C=256, N=1024)):
  q = Wq@X + bq ; k = Wk@X + bk ; v = Wv@X + bv      (per-pixel linear)
  S[n,m] = sum_d q[d,n]k[d,m] / sqrt(32)  (per head)
  P = softmax_m(S) ; O[d,n] = sum_m P[n,m] v[d,m] ; out = Wo@O + bo + X

Bias algebra (folded on host, exact):
  - bk contributes q^T bk, constant along softmax axis -> drops.
  - bq contributes (bq^T k_raw)[m]: folded as an augmented K-hat row
    u_h = Wk_h^T bq_h / sqrt32, matched by a ones-row in Q-hat.
  - bv contributes bv (softmax weights sum to 1) -> xpb = x + (Wo@bv + bo).
  - 1/sqrt(32) folded into Wq-hat and the u rows.

Layouts: engine APs must sit at base partitions {0,32,64}, forcing the
64-row head pitch and DMA-based esum/O remaps:
  Qh/Kh: 8 head-slabs (head h rows 64h..64h+33; Q row 32 = ones memset,
         K row 32 = u row), 4 SBUF tiles [128,1024] bf16.
  S^T per (head-pair, m-chunk, n-half): psum [128, 2x512]; exp on ACT ->
  E [128,1024] bf16 (64 serial exps = the ~65us ACT floor).
  VH[mc] [128, 8*33] bf16: 32 V^T cols + a ones col per head, so the AV
  matmul also accumulates the softmax denominator (psO rows 32/96) free.

Schedule (vs the 135.7us v1 baseline):
  - All matmul operands bf16; PE work ~158K cycles, ACT ~65us -- the two
    jointly pace a steady state of 2 exps per (pair, m-chunk).
  - Scores double-buffered in a 4-bank psum rotation; projections /
    V^T / broadcast / output-projection transients live in a separate
    2-bank aux slab so their allocation never stalls the score stream;
    remaining 2 banks accumulate the pair's AV.
  - All non-score PE work is chunked to <=4 matmuls and spread across the
    mc slots where the exp backlog absorbs it.
  - Per-pair softmax denominators (ESUM[p]), reciprocal_approx_fast, and
    a split broadcast matmul let the pair-2 normalization half run
    mid-stream; only pair 3's half + output projection kc=1 trail.
  - First-wave DMA issue is spread over sync/scalar/gpsimd sequencers;
    first scores fire from half-granular chunk-0 projection copies.
"""

import math

import numpy as np
import ml_dtypes

import concourse.bass as bass
import concourse.mybir as mybir
import concourse.tile as tile
from concourse import bacc
from concourse.bass_utils import run_bass_kernel_spmd

F32 = mybir.dt.float32
F32R = mybir.dt.float32r
BF16 = mybir.dt.bfloat16
EXP = mybir.ActivationFunctionType.Exp

NH = 8          # heads
HD = 32         # head dim
C = 256         # channels
N = 1024        # H*W
NCORES = 8

_NC = None
LAST_RESULTS = None


def _emit(tc, io):
    nc = tc.nc
    import contextlib

    ctx = contextlib.ExitStack()
    with ctx:
        pers = ctx.enter_context(tc.tile_pool(name="pers", bufs=1))
        etp = ctx.enter_context(tc.tile_pool(name="etp", bufs=5))
        psp = ctx.enter_context(tc.tile_pool(name="psp", bufs=2, space="PSUM"))

        def ptile(name, shape, dtype=F32):
            return pers.tile(shape, dtype, tag=name, name=name)

        # ---------------- persistent tiles / input DMA ----------------
        XB = [ptile(f"XB{i}", [128, N], BF16) for i in range(2)]
        WQ = [ptile(f"WQ{i}", [128, 512], BF16) for i in range(2)]
        WK = [ptile(f"WK{i}", [128, 512], BF16) for i in range(2)]
        WV = [ptile(f"WV{i}", [128, C], BF16) for i in range(2)]
        WO = [ptile(f"WO{i}", [128, C], BF16) for i in range(2)]
        XPB = [ptile(f"XPB{i}", [128, N]) for i in range(2)]
        OHP = [ptile(f"OHP{i}", [2, 128], F32R) for i in range(2)]
        # first-wave loads in criticality order, issue spread over engine
        # sequencers (each DIRECT2D descriptor-gen is ~0.6us, serial per
        # engine -- single-engine issue delays the last load by ~5us)
        nc.sync.dma_start(WQ[0][:], io["wq"][0:128, :])
        nc.sync.dma_start(WQ[1][:], io["wq"][128:256, :])
        for i in range(2):
            nc.sync.dma_start(XB[i][:], io["xb"][i * 128 : (i + 1) * 128, :])
        for i in range(2):
            nc.scalar.dma_start(WK[i][:], io["wk"][i * 128 : (i + 1) * 128, :])
        for i in range(2):
            nc.gpsimd.dma_start(WV[i][:], io["wv"][i * 128 : (i + 1) * 128, :])
        for i in range(2):
            nc.gpsimd.dma_start(OHP[i][:], io["oh"][2 * i : 2 * i + 2, :])

        # warm the ACT exp table right after the first weight DMA lands
        # (output is never read; input is just convenient loaded data)
        warm = ptile("warm", [1, 32])
        nc.scalar.activation(warm[:], WQ[0][0:1, 0:32], EXP)

        Qh = [ptile(f"Qh{t}", [128, N], BF16) for t in range(4)]
        Kh = [ptile(f"Kh{t}", [128, N], BF16) for t in range(4)]
        VH = [ptile(f"VH{mc}", [128, NH * 33], BF16) for mc in range(NH)]
        O1U = [ptile(f"O1U{t}", [128, N]) for t in range(2)]
        ESUM = [ptile(f"ESUM{p}", [2, N]) for p in range(4)]
        O1 = [ptile(f"O1{t}", [128, N], BF16) for t in range(2)]
        PART = [ptile(f"PART{i}", [128, N]) for i in range(2)]
        OUTF = [ptile(f"OUTF{i}", [128, N]) for i in range(2)]

        # ---------------- building blocks ----------------
        def qk_pp(t, w, tag, bufs, nm):
            """Allocate the projection psum for one Q-hat/K-hat chunk and
            return per-half emitters (2 matmuls + trailing bf16 copy)."""
            pp = psp.tile([128, N], F32, tag=tag, bufs=bufs, name=f"pp{nm}{t}")
            dst = Qh if nm == "q" else Kh

            def half(jn):
                js = slice(jn * 512, (jn + 1) * 512)
                for kc in range(2):
                    nc.tensor.matmul(
                        pp[:, js],
                        w[kc][:, t * 128 : (t + 1) * 128],
                        XB[kc][:, js],
                        start=(kc == 0),
                        stop=(kc == 1),
                    )
                nc.vector.tensor_copy(dst[t][:, js], pp[:, js])
                if nm == "q":
                    for s in range(2):
                        nc.vector.memset(Qh[t][64 * s + 32 : 64 * s + 33, js], 1.0)

            return half

        def qk_units(t):
            """4 deferred units (Qh half0/1, Kh half0/1) for chunk t>0."""
            state = {}

            def unit(nm, w, jn):
                def run():
                    if nm not in state:
                        state[nm] = qk_pp(t, w, "aux", 1, nm)
                    state[nm](jn)

                return run

            return [
                unit("q", WQ, 0),
                unit("q", WQ, 1),
                unit("k", WK, 0),
                unit("k", WK, 1),
            ]

        def pv_pack(i):
            """V^T for m-chunks 2i and 2i+1 in one 1-bank psum alloc."""
            pvp = psp.tile([128, 512], F32, tag="aux", bufs=1, name=f"pvp{i}")
            for k in range(2):
                mc = 2 * i + k
                for kc in range(2):
                    nc.tensor.matmul(
                        pvp[:, k * 256 : (k + 1) * 256],
                        XB[kc][:, mc * 128 : (mc + 1) * 128],
                        WV[kc][:],
                        start=(kc == 0),
                        stop=(kc == 1),
                    )
            for k in range(2):
                mc = 2 * i + k
                nc.vector.memset(VH[mc][:], 1.0)
                vh3 = VH[mc].rearrange("p (h c) -> p h c", c=33)
                nc.vector.tensor_copy(
                    vh3[:, :, 0:32],
                    pvp[:, k * 256 : (k + 1) * 256].rearrange(
                        "p (h d) -> p h d", d=32
                    ),
                )

        def score_mms(p, mc, jn):
            ps = psp.tile([128, N], F32, tag="ps", bufs=2, name=f"ps{p}_{mc}_{jn}")
            for hh in range(2):
                base = 64 * hh
                nc.tensor.matmul(
                    ps[:, hh * 512 : (hh + 1) * 512],
                    Kh[p][base : base + 33, mc * 128 : (mc + 1) * 128],
                    Qh[p][base : base + 33, jn * 512 : (jn + 1) * 512],
                    start=True,
                    stop=True,
                )
            return ps

        def exp_op(p, mc, jn, ps):
            et = etp.tile([128, N], BF16, tag="et", name=f"et{p}_{mc}_{jn}")
            nc.scalar.activation(et[:], ps[:], EXP)
            return et

        def av_mms(p, mc, jn, et, psO):
            for hh in range(2):
                h = 2 * p + hh
                nc.tensor.matmul(
                    psO[jn][64 * hh : 64 * hh + 33, :],
                    VH[mc][:, 33 * h : 33 * h + 33],
                    et[:, hh * 512 : (hh + 1) * 512],
                    start=(mc == 0),
                    stop=(mc == 7),
                    tile_position=(0, 64 * hh),
                    skip_group_check=True,
                )

        def pair_out(p, psO):
            """Copy psO to SBUF; DMA O rows into O1U and esum rows into
            ESUM[p].  Esum rows go first (they gate the recip); for the last
            pair the copies run on ACT -- idle once the exps are done -- and
            the DMA issue is spread over the sync/gpsimd sequencers."""
            t = p // 2
            copy = nc.scalar.copy if p == 3 else nc.vector.tensor_copy
            for jn in range(2):
                js = slice(jn * 512, (jn + 1) * 512)
                ost = etp.tile([97, 512], F32, tag="ost", bufs=4, name=f"ost{p}_{jn}")
                copy(ost[32:33, :], psO[jn][32:33, :])
                copy(ost[64:97, :], psO[jn][64:97, :])
                for hh in range(2):
                    nc.sync.dma_start(
                        ESUM[p][hh : hh + 1, js],
                        ost[64 * hh + 32 : 64 * hh + 33, :],
                    )
                copy(ost[0:33, :], psO[jn][0:33, :])
                for hh in range(2):
                    h = 2 * p + hh
                    r = 32 * (h % 4)
                    nc.gpsimd.dma_start(
                        O1U[t][r : r + 32, js], ost[64 * hh : 64 * hh + 32, :]
                    )

        prn = {}

        def norm_half(t, parity):
            """recip + broadcast-matmul contribution of pair 2t+parity into
            pr[t].  The pair-even half of tile 1 runs mid-stream so only the
            pair-odd half is left in the tail chain."""
            p = 2 * t + parity
            if t not in prn:
                prn[t] = psp.tile([128, N], F32, tag="aux", bufs=1, name=f"prn{t}")
            RECIP = etp.tile([2, N], F32, tag="recip", bufs=2, name=f"recip{p}")
            RECIPR = etp.tile([2, N], F32R, tag="recipr", bufs=2, name=f"recipr{p}")
            for jn in range(2):
                js = slice(jn * 512, (jn + 1) * 512)
                with nc.allow_low_precision("softmax denom recip (~2e-6 rel)"):
                    nc.vector.reciprocal_approx_fast(RECIP[:, js], ESUM[p][:, js])
                nc.vector.tensor_copy(RECIPR[:, js], RECIP[:, js])
                nc.tensor.matmul(
                    prn[t][:, js],
                    OHP[parity][:],
                    RECIPR[:, js],
                    start=(parity == 0),
                    stop=(parity == 1),
                    skip_group_check=True,
                )

        def norm_finish(t, interleave_po=False):
            """O1 = O1U * pr; optionally chase each n-half with its kc=1
            output projection (tail pipelining)."""
            for jn in range(2):
                js = slice(jn * 512, (jn + 1) * 512)
                nc.vector.tensor_mul(O1[t][:, js], O1U[t][:, js], prn[t][:, js])
                if interleave_po:
                    for mo in range(2):
                        po_unit(1, mo, jn, "ps")

        def po_unit(kc, mo, jn, tag):
            js = slice(jn * 512, (jn + 1) * 512)
            po = psp.tile(
                [128, 512], F32, tag=tag, bufs=1 if tag == "aux" else 2,
                name=f"po{kc}_{mo}_{jn}",
            )
            nc.tensor.matmul(
                po[:],
                WO[kc][:, mo * 128 : (mo + 1) * 128],
                O1[kc][:, js],
                start=True,
                stop=True,
            )
            if kc == 0:
                nc.vector.tensor_add(PART[mo][:, js], po[:], XPB[mo][:, js])
            else:
                nc.vector.tensor_add(OUTF[mo][:, js], po[:], PART[mo][:, js])
                nc.gpsimd.dma_start(
                    io["out"][mo * 128 : (mo + 1) * 128, js], OUTF[mo][:, js]
                )

        # ---------------- emission schedule ----------------
        def new_psO(p):
            return [
                psp.tile([97, 512], F32, tag="psO", bufs=2, name=f"psO{p}_{jn}")
                for jn in range(2)
            ]

        psO = new_psO(0)

        # chunk 0 projections upfront on the (free) score-psum rotation, as
        # 1-bank per-half allocs interleaved with the first score matmuls so
        # the first exp fires as soon as the h0 copies land
        def qk0_half(w, nm, jn):
            pp = psp.tile([128, 512], F32, tag="ps", bufs=2, name=f"pp0{nm}{jn}")
            dst = Qh if nm == "q" else Kh
            js = slice(jn * 512, (jn + 1) * 512)
            for kc in range(2):
                nc.tensor.matmul(
                    pp[:], w[kc][:, 0:128], XB[kc][:, js],
                    start=(kc == 0), stop=(kc == 1),
                )
            nc.vector.tensor_copy(dst[0][:, js], pp[:])
            if nm == "q":
                for s in range(2):
                    nc.vector.memset(Qh[0][64 * s + 32 : 64 * s + 33, js], 1.0)

        qk0_half(WQ, "q", 0)
        qk0_half(WK, "k", 0)
        ps000 = score_mms(0, 0, 0)
        qk0_half(WQ, "q", 1)
        qk0_half(WK, "k", 1)
        ps001 = score_mms(0, 0, 1)

        qk1, qk2, qk3 = qk_units(1), qk_units(2), qk_units(3)
        late_dma = lambda: [
            (
                nc.sync.dma_start(WO[i][:], io["wo"][i * 128 : (i + 1) * 128, :]),
                nc.sync.dma_start(XPB[i][:], io["xpb"][i * 128 : (i + 1) * 128, :]),
            )
            for i in range(2)
        ]
        deferred = {
            (0, 0): [lambda: pv_pack(0)],
            (0, 1): [lambda: pv_pack(1)],
            (0, 2): [lambda: pv_pack(2)],
            (0, 3): [lambda: pv_pack(3)],
            (0, 4): [qk1[0]],
            (0, 5): [qk1[1]],
            (0, 6): [qk1[2]],
            (0, 7): [qk1[3]],
            (1, 0): [qk2[0]],
            (1, 1): [qk2[1]],
            (1, 2): [qk2[2]],
            (1, 3): [qk2[3]],
            (1, 4): [late_dma, qk3[0]],
            (1, 5): [qk3[1]],
            (1, 6): [qk3[2]],
            (1, 7): [qk3[3]],
            (2, 1): [lambda: norm_half(0, 0)],
            (2, 2): [lambda: (norm_half(0, 1), norm_finish(0))],
            (2, 3): [lambda: po_unit(0, 0, 0, "aux")],
            (3, 2): [lambda: norm_half(1, 0)],
            (2, 4): [lambda: po_unit(0, 1, 0, "aux")],
            (2, 5): [lambda: po_unit(0, 0, 1, "aux")],
            (2, 6): [lambda: po_unit(0, 1, 1, "aux")],
        }

        for p in range(4):
            if p > 0:
                psO = new_psO(p)
            for mc in range(8):
                if (p, mc) == (0, 0):
                    pss = [ps000, ps001]
                else:
                    pss = [score_mms(p, mc, jn) for jn in range(2)]
                ets = []
                for jn in range(2):
                    ets.append(exp_op(p, mc, jn, pss[jn]))
                for fn in deferred.get((p, mc), ()):
                    fn()
                for jn in range(2):
                    av_mms(p, mc, jn, ets[jn], psO)
            pair_out(p, psO)
        norm_half(1, 1)
        norm_finish(1, interleave_po=True)


def build_nc():
    nc = bacc.Bacc("TRN2", target_bir_lowering=False, debug=False)
    io = {}
    for name, shape, dt_ in [
        ("xb", (C, N), BF16),
        ("wq", (C, 512), BF16),
        ("wk", (C, 512), BF16),
        ("wv", (C, C), BF16),
        ("wo", (C, C), BF16),
        ("xpb", (C, N), F32),
        ("oh", (4, 128), F32R),
    ]:
        io[name] = nc.dram_tensor(name, shape, dt_, kind="ExternalInput").ap()
    io["out"] = nc.dram_tensor("out", (C, N), F32, kind="ExternalOutput").ap()
    with tile.TileContext(nc) as tc:
        _emit(tc, io)
    nc.finalize()
    return nc


def host_prep(x, Wq, bq, Wk, bk, Wv, bv, Wo, bo):
    """Build per-core input maps (numpy only)."""
    bf16 = ml_dtypes.bfloat16
    x = np.ascontiguousarray(np.asarray(x, np.float32))
    Wq, bq = np.asarray(Wq, np.float32), np.asarray(bq, np.float32)
    Wk = np.asarray(Wk, np.float32)
    Wv, bv = np.asarray(Wv, np.float32), np.asarray(bv, np.float32)
    Wo, bo = np.asarray(Wo, np.float32), np.asarray(bo, np.float32)
    s = 1.0 / math.sqrt(HD)

    wq_hat = np.zeros((C, 512), np.float32)
    wk_hat = np.zeros((C, 512), np.float32)
    for h in range(NH):
        hs = slice(HD * h, HD * (h + 1))
        wq_hat[:, 64 * h : 64 * h + 32] = Wq[hs, :].T * s
        wk_hat[:, 64 * h : 64 * h + 32] = Wk[hs, :].T
        wk_hat[:, 64 * h + 32] = (Wk[hs, :].T @ bq[hs]) * s

    # oh rows 0-1: pair-even stationary (recip row r -> O1U rows 32r..);
    # rows 2-3: pair-odd stationary (-> O1U rows 64+32r..)
    oh = np.zeros((4, 128), np.float32)
    for parity in range(2):
        for r in range(2):
            oh[2 * parity + r, 64 * parity + 32 * r : 64 * parity + 32 * r + 32] = 1.0

    bo2 = Wo @ bv + bo

    common = {
        "wq": wq_hat.astype(bf16),
        "wk": wk_hat.astype(bf16),
        "wv": np.ascontiguousarray(Wv.T).astype(bf16),
        "wo": np.ascontiguousarray(Wo.T).astype(bf16),
        "oh": oh,
    }

    B = x.shape[0]
    in_maps = []
    for b in range(B):
        xb = np.ascontiguousarray(x[b].reshape(C, N))
        m = dict(common)
        m["xb"] = xb.astype(bf16)
        m["xpb"] = np.ascontiguousarray(xb + bo2[:, None])
        in_maps.append(m)
    return in_maps


def kernel(x, Wq, bq, Wk, bk, Wv, bv, Wo, bo):
    global _NC, LAST_RESULTS
    if _NC is None:
        _NC = build_nc()
    in_maps = host_prep(x, Wq, bq, Wk, bk, Wv, bv, Wo, bo)
    res = run_bass_kernel_spmd(_NC, in_maps, core_ids=list(range(NCORES)))
    LAST_RESULTS = res
    out = np.stack([r["out"] for r in res.results], axis=0)
    return out.reshape(NCORES, C, 32, 32).astype(np.float32)


if __name__ == "__main__":
    rng = np.random.default_rng(0)
    ins = {
        "x": rng.standard_normal((8, C, 32, 32), dtype=np.float32),
        "Wq": rng.standard_normal((C, C), dtype=np.float32) / 16,
        "bq": rng.standard_normal(C).astype(np.float32) * 0.01,
        "Wk": rng.standard_normal((C, C), dtype=np.float32) / 16,
        "bk": rng.standard_normal(C).astype(np.float32) * 0.01,
        "Wv": rng.standard_normal((C, C), dtype=np.float32) / 16,
        "bv": rng.standard_normal(C).astype(np.float32) * 0.01,
        "Wo": rng.standard_normal((C, C), dtype=np.float32) / 16,
        "bo": rng.standard_normal(C).astype(np.float32) * 0.01,
    }
    out = kernel(**ins)
    print("out", out.shape, out.dtype, float(np.abs(out).mean()))



# revision 4
# speedup vs baseline: 1.0979x; 1.0979x over previous
"""Multi-head self-attention Trainium2 kernel (Bass/Tile), v7.

Problem: x:(8,256,32,32), 8 heads, head_dim=32, N=H*W=1024.
Sharding: data-parallel over batch B=8 -> one batch element per NeuronCore.

Design vs v6 (135.8us local):
  - 32-pitch head layout: Qh/Kh [128,1024] x 2 tiles, head 4t+j at
    partitions 32j.  Halves Q/K projection streaming (no zero columns).
  - q-bias via DVE tensor_scalar_add (bq folded, scaled); no u-row/ones-row.
    S = (Wq x * s + bq s)^T (Wk x): bk/bq constants drop under softmax.
  - Steady state is ACT-paced: per (pair, mc): 2 score-pair MMs (K=32,
    2-way row-group concurrent), 2 exps [128,1024], 1 AV pair (M=33,
    2-way col-group concurrent, N=1024 streams).  PE/slot ~= 1.0us cold
    < 1.11us exp, so ACT never waits on PE even at K=4/8.
  - Garbage-matmul warm-up at t=0 (scratch SBUF) warms HAM during input DMA
    (an exp-paced stream never warms it; steady state runs at 1.2GHz).
  - Projections: K0/Q0 half-granular on the ps rotation feed the first
    score; V and K1/Q1 injected into pair-0 slot shadows.
  - PSUM: ps 2x[128,1024] (score double-buffer, 4 banks) + psO 2x[98,512]
    (per-jn AV accum so next pair's AV never waits the drain; esum rows
    ride at partitions 32/96 via VH ones-cols) + aux [128,*] (proj/prn/po
    transients, 2 banks) = 8 banks.  MM outputs are capped at 512 f32
    cols (one-bank ISA rule); concurrent same-bank MM writes wedge the
    device.
  - Per-pair epilogue: DVE copies psO -> OST (partition-aligned, no
    remap), DMA-gathers esum rows to base-0 ESUM (recip_approx_fast
    miscomputes on base!=0 APs), recip+bf16 cast, per-pair broadcast MM
    (prn), O1_p = OST*prn, pair-split output projection (WOP_p has zero
    rows at esum/junk positions so junk*0 stays clean), PART[mo]
    accumulated on DVE.  Pair 3 runs this chain jn-pipelined, copies
    split ACT/DVE, with quarter-wise output DMA on separate queues.
"""

import math

import numpy as np
import ml_dtypes

import concourse.bass as bass
import concourse.mybir as mybir
import concourse.tile as tile
from concourse import bacc
from concourse.bass_utils import run_bass_kernel_spmd

F32 = mybir.dt.float32
BF16 = mybir.dt.bfloat16
EXP = mybir.ActivationFunctionType.Exp

NH = 8
HD = 32
C = 256
N = 1024
NCORES = 8

_NC = None
LAST_RESULTS = None
import os as _os
KEEPALIVE = int(_os.environ.get("BASS_KEEPALIVE", "0"))


def _emit(tc, io):
    nc = tc.nc
    import contextlib

    ctx = contextlib.ExitStack()
    with ctx:
        pers = ctx.enter_context(tc.tile_pool(name="pers", bufs=1))
        etp = ctx.enter_context(tc.tile_pool(name="etp", bufs=3))
        psp = ctx.enter_context(tc.tile_pool(name="psp", bufs=1, space="PSUM"))

        def ptile(name, shape, dtype=F32):
            return pers.tile(shape, dtype, tag=name, name=name)

        # ---------------- persistent tiles ----------------
        XB = [ptile(f"XB{i}", [128, N], BF16) for i in range(2)]
        WQ = [ptile(f"WQ{i}", [128, C], BF16) for i in range(2)]
        WK = [ptile(f"WK{i}", [128, C], BF16) for i in range(2)]
        WV = [ptile(f"WV{i}", [128, C], BF16) for i in range(2)]
        WOP = [ptile(f"WOP{p}", [97, C], BF16) for p in range(4)]
        BQ = ptile("BQ", [128, 2])
        OHB = ptile("OHB", [2, 97], BF16)
        XPB = [ptile(f"XPB{i}", [128, N]) for i in range(2)]
        Qh = [ptile(f"Qh{t}", [128, N], BF16) for t in range(2)]
        Kh = [ptile(f"Kh{t}", [128, N], BF16) for t in range(2)]
        VH = [ptile(f"VH{mc}", [128, NH * 33], BF16) for mc in range(NH)]
        OST = [ptile(f"OST{p}", [97, N]) for p in range(4)]
        ESUM = [ptile(f"ESUM{p}", [2, N]) for p in range(4)]
        RECB = [ptile(f"RECB{p}", [2, N], BF16) for p in range(4)]
        O1 = [ptile(f"O1{p}", [97, N], BF16) for p in range(4)]
        PART = [ptile(f"PART{mo}", [128, N]) for mo in range(2)]
        SCR = ptile("SCR", [128, 640], BF16)

        # ---------------- warm-up + input DMA ----------------
        # garbage matmuls on scratch SBUF warm the PE HAM clock-gate to
        # 8/8 while the first-wave DMAs land (nothing depends on them)
        nc.vector.memset(SCR[:], 0.25)
        # OST rows 33-63 are never written by the psO drains; zero them once
        # so the O1 multiply reads initialized data (prn rows are 0 there)
        for p in range(4):
            nc.vector.memset(OST[p][32:64, :], 0.0)
        wup = psp.tile([128, N], F32, tag="aux", bufs=1, name="wup")
        for i in range(14):
            nc.tensor.matmul(
                wup[:, 0:512] if i % 2 == 0 else wup[:, 512:1024],
                SCR[:, 0:128], SCR[:, 128:640],
                start=True, stop=True,
            )
        nc.vector.tensor_copy(SCR[0:1, 0:32], wup[0:1, 0:32])

        # first-wave loads, issue spread over engine sequencers
        nc.sync.dma_start(XB[0][:], io["xb"][0:128, :])
        nc.sync.dma_start(XB[1][:], io["xb"][128:256, :])
        for i in range(2):
            nc.scalar.dma_start(WK[i][:], io["wk"][i * 128 : (i + 1) * 128, :])
        for i in range(2):
            nc.gpsimd.dma_start(WQ[i][:], io["wq"][i * 128 : (i + 1) * 128, :])
        nc.gpsimd.dma_start(BQ[:], io["bqc"])
        for i in range(2):
            nc.scalar.dma_start(WV[i][:], io["wv"][i * 128 : (i + 1) * 128, :])
        nc.gpsimd.dma_start(OHB[:], io["ohb"])

        # warm the ACT exp table while weights land
        warm = ptile("warm", [1, 32])
        nc.scalar.activation(warm[:], WK[0][0:1, 0:32], EXP)

        def late_dma():
            for p in range(4):
                nc.sync.dma_start(WOP[p][:], io["wop"][97 * p : 97 * p + 97, :])
            for i in range(2):
                nc.sync.dma_start(XPB[i][:], io["xpb"][i * 128 : (i + 1) * 128, :])

        # ---------------- building blocks ----------------
        def qk_half(t, jn, w, dst, bias, tag="aux"):
            """Half (512 pixels) of one 128-row chunk of the Q/K projection.
            Bias added on the psum->SBUF copy for Q."""
            js = slice(jn * 512, (jn + 1) * 512)
            pp = psp.tile(
                [128, 512], F32, tag=tag, bufs=1 if tag == "aux" else 2,
                name=f"pp{dst is Qh}{t}_{jn}",
            )
            for kc in range(2):
                nc.tensor.matmul(
                    pp[:], w[kc][:, t * 128 : (t + 1) * 128], XB[kc][:, js],
                    start=(kc == 0), stop=(kc == 1),
                )
            if bias is not None:
                nc.vector.tensor_scalar_add(dst[t][:, js], pp[:], bias[:, t : t + 1])
            else:
                nc.vector.tensor_copy(dst[t][:, js], pp[:])

        def pv_pack(i):
            """V^T for m-chunks 2i, 2i+1 in one aux psum alloc."""
            pvp = psp.tile([128, 512], F32, tag="aux", bufs=1, name=f"pvp{i}")
            for k in range(2):
                mc = 2 * i + k
                for kc in range(2):
                    nc.tensor.matmul(
                        pvp[:, k * 256 : (k + 1) * 256],
                        XB[kc][:, mc * 128 : (mc + 1) * 128],
                        WV[kc][:],
                        start=(kc == 0), stop=(kc == 1),
                    )
            for k in range(2):
                mc = 2 * i + k
                vh3 = VH[mc].rearrange("p (h c) -> p h c", c=33)
                pv3 = pvp[:, k * 256 : (k + 1) * 256].rearrange(
                    "p (h d) -> p h d", d=32
                )
                nc.vector.memset(VH[mc][:], 1.0)
                nc.vector.tensor_copy(vh3[:, :, 0:32], pv3[:, :, :])

        def score_mms(p, mc, jn):
            t, half = p // 2, p % 2
            ps = psp.tile([128, N], F32, tag="ps", bufs=2, name=f"ps{p}_{mc}_{jn}")
            for hh in range(2):
                base = 64 * half + 32 * hh
                nc.tensor.matmul(
                    ps[:, hh * 512 : (hh + 1) * 512],
                    Kh[t][base : base + 32, mc * 128 : (mc + 1) * 128],
                    Qh[t][base : base + 32, jn * 512 : (jn + 1) * 512],
                    start=True, stop=True,
                    tile_position=(base, 0),
                    skip_group_check=True,
                )
            return ps

        def exp_op(p, mc, jn, ps, et):
            nc.scalar.activation(et[:], ps[:], EXP)

        def av_mms(p, mc, jn, et, psO):
            for hh in range(2):
                h = 2 * p + hh
                nc.tensor.matmul(
                    psO[jn][64 * hh : 64 * hh + 33, :],
                    VH[mc][:, 33 * h : 33 * h + 33],
                    et[:, hh * 512 : (hh + 1) * 512],
                    start=(mc == 0), stop=(mc == 7),
                    tile_position=(0, 64 * hh),
                    skip_group_check=True,
                )

        def drain_jn(p, jn, psO, split_engines=False):
            """psO[jn] -> OST columns (partition-aligned) + esum DMA gather.
            Tiny esum-row copies go first so the gather DMAs fire early;
            for the tail-critical last drain the copies split across ACT
            (idle by then) and DVE."""
            js = slice(jn * 512, (jn + 1) * 512)
            copyA = nc.scalar.copy if split_engines else nc.vector.tensor_copy
            copyA(OST[p][32:33, js], psO[jn][32:33, :])
            nc.vector.tensor_copy(OST[p][96:97, js], psO[jn][96:97, :])
            nc.sync.dma_start(ESUM[p][0:1, js], OST[p][32:33, js])
            nc.scalar.dma_start(ESUM[p][1:2, js], OST[p][96:97, js])
            copyA(OST[p][0:32, js], psO[jn][0:32, :])
            nc.vector.tensor_copy(OST[p][64:96, js], psO[jn][64:96, :])

        def recip_jn(p, jn, cast_on_act=False):
            js = slice(jn * 512, (jn + 1) * 512)
            RECF = etp.tile([2, 512], F32, tag="recf", bufs=2, name=f"recf{p}{jn}")
            with nc.allow_low_precision("softmax denom recip (~1e-3 rel)"):
                nc.vector.reciprocal_approx_fast(RECF[:], ESUM[p][:, js])
            if cast_on_act:
                nc.scalar.copy(RECB[p][:, js], RECF[:])
            else:
                nc.vector.tensor_copy(RECB[p][:, js], RECF[:])

        def recip_pair(p):
            for jn in range(2):
                recip_jn(p, jn)

        prns = {}

        def prn_mms(p, jn=None, tag="aux"):
            """prn_p = OHB^T @ RECB[p] broadcast into 32-row blocks."""
            if p not in prns:
                prns[p] = psp.tile(
                    [128, N], F32, tag=tag, bufs=2 if tag == "ps" else 1,
                    name=f"prn{p}",
                )
            prn = prns[p]
            for j in ((0, 1) if jn is None else (jn,)):
                js = slice(j * 512, (j + 1) * 512)
                nc.tensor.matmul(
                    prn[0:97, js], OHB[:], RECB[p][:, js],
                    start=True, stop=True,
                    skip_group_check=True,
                )

        def norm_mul(p, jn=None):
            prn = prns[p]
            if jn is None:
                nc.vector.tensor_mul(O1[p][:], OST[p][:], prn[0:97, :])
            else:
                js = slice(jn * 512, (jn + 1) * 512)
                nc.vector.tensor_mul(O1[p][:, js], OST[p][:, js], prn[0:97, js])

        OUT_QUEUES = [nc.sync, nc.scalar, nc.gpsimd, nc.sync]

        po3 = {}

        def po_unit(p, mo, tag="aux", jn=None):
            """Pair p's contribution to output chunk mo; PART accumulates
            on DVE (PART[mo] starts as XPB[mo] + pair0).  For the final
            pair (jn-split) the adds/DMAs go out quarter-wise on separate
            queues."""
            if p == 3:
                if mo not in po3:
                    po3[mo] = psp.tile(
                        [128, N], F32, tag=tag, bufs=2, name=f"po3_{mo}"
                    )
                po = po3[mo]
                js = slice(jn * 512, (jn + 1) * 512)
                nc.tensor.matmul(
                    po[:, js], WOP[p][:, mo * 128 : (mo + 1) * 128],
                    O1[p][:, js],
                    start=True, stop=True,
                )
                nc.vector.tensor_add(PART[mo][:, js], po[:, js], PART[mo][:, js])
                OUT_QUEUES[2 * mo + jn].dma_start(
                    io["out"][mo * 128 : (mo + 1) * 128, js],
                    PART[mo][:, js],
                )
                return
            raise AssertionError("mid-stream pairs use po_half")

        pos = {}

        def po_half(p, mo, jn):
            """One jn-half of pair p's output-projection contribution."""
            if (p, mo) not in pos:
                pos[(p, mo)] = psp.tile(
                    [128, N], F32, tag="aux", bufs=1, name=f"po{p}_{mo}"
                )
            po = pos[(p, mo)]
            js = slice(jn * 512, (jn + 1) * 512)
            nc.tensor.matmul(
                po[:, js], WOP[p][:, mo * 128 : (mo + 1) * 128],
                O1[p][:, js],
                start=True, stop=True,
            )
            if p == 0:
                nc.vector.tensor_add(PART[mo][:, js], po[:, js], XPB[mo][:, js])
            else:
                nc.vector.tensor_add(PART[mo][:, js], po[:, js], PART[mo][:, js])

        # ---------------- emission schedule ----------------
        # first-score prerequisites, half-granular on the ps rotation
        qk_half(0, 0, WK, Kh, None, tag="ps")
        qk_half(0, 0, WQ, Qh, BQ, tag="ps")

        # deferred PE work injected into slot shadows
        deferred = {
            (0, 0): [lambda: pv_pack(0)],
            (0, 1): [lambda: pv_pack(1)],
            (0, 2): [lambda: pv_pack(2)],
            (0, 3): [lambda: pv_pack(3)],
            (0, 4): [lambda: qk_half(1, 0, WK, Kh, None)],
            (0, 5): [late_dma, lambda: qk_half(1, 0, WQ, Qh, BQ)],
            (0, 6): [lambda: qk_half(1, 1, WK, Kh, None)],
            (0, 7): [lambda: qk_half(1, 1, WQ, Qh, BQ)],
        }
        for q in range(3):
            deferred[(q + 1, 1)] = [lambda q=q: prn_mms(q, jn=0)]
            deferred[(q + 1, 2)] = [lambda q=q: (prn_mms(q, jn=1),
                                                 norm_mul(q))]
            deferred[(q + 1, 3)] = [lambda q=q: po_half(q, 0, 0)]
            deferred[(q + 1, 4)] = [lambda q=q: po_half(q, 0, 1)]
            deferred[(q + 1, 5)] = [lambda q=q: po_half(q, 1, 0)]
            deferred[(q + 1, 6)] = [lambda q=q: po_half(q, 1, 1)]

        tail3 = []
        for p in range(4):
            psO = [
                psp.tile([98, 512], F32, tag="psO", bufs=2, name=f"psO{p}_{jn}")
                for jn in range(2)
            ]
            for mc in range(8):
                ets = []
                for jn in range(2):
                    if (p, mc, jn) == (0, 0, 1):
                        # rest of chunk-0 projections right behind score 0
                        qk_half(0, 1, WK, Kh, None, tag="ps")
                        qk_half(0, 1, WQ, Qh, BQ, tag="ps")
                    ps = score_mms(p, mc, jn)
                    et = etp.tile([128, N], BF16, tag="et", name=f"et{p}_{mc}_{jn}")
                    exp_op(p, mc, jn, ps, et)
                    ets.append(et)
                for fn in deferred.get((p, mc), ()):
                    fn()
                for jn in range(2):
                    av_mms(p, mc, jn, ets[jn], psO)
                    if mc == 7 and p < 3:
                        drain_jn(p, jn, psO)
                    elif mc == 7:
                        # jn-pipelined tail, DVE/ACT balanced: esum copies +
                        # gather DMAs fire first; jn1's early copies run on
                        # the just-idle ACT; the recip DMA-wait is filled
                        # with block copies; casts go to ACT so DVE keeps
                        # only recip/mul/add on the critical chain.
                        js = slice(jn * 512, (jn + 1) * 512)
                        copyA = nc.scalar.copy if jn else nc.vector.tensor_copy
                        copyA(OST[3][32:33, js], psO[jn][32:33, :])
                        nc.vector.tensor_copy(OST[3][96:97, js], psO[jn][96:97, :])
                        nc.sync.dma_start(ESUM[3][0:1, js], OST[3][32:33, js])
                        nc.scalar.dma_start(ESUM[3][1:2, js], OST[3][96:97, js])
                        copyA(OST[3][0:32, js], psO[jn][0:32, :])
                        nc.vector.tensor_copy(OST[3][64:96, js], psO[jn][64:96, :])
                        tail3.append(jn)

                def run_tail3():
                    for jn in tail3:
                        recip_jn(3, jn, cast_on_act=True)
                        prn_mms(3, jn=jn, tag="aux")
                        norm_mul(3, jn)
                        po_unit(3, 0, tag="ps", jn=jn)
                        po_unit(3, 1, tag="ps", jn=jn)
                    tail3.clear()

                if p == 3 and mc == 7:
                    run_tail3()
                for _ in range(KEEPALIVE):
                    nc.tensor.ldweights(SCR[:, 0:128])
            if p < 3:
                recip_pair(p)


def build_nc():
    nc = bacc.Bacc("TRN2", target_bir_lowering=False, debug=False)
    io = {}
    for name, shape, dt_ in [
        ("xb", (C, N), BF16),
        ("wq", (C, C), BF16),
        ("wk", (C, C), BF16),
        ("wv", (C, C), BF16),
        ("wop", (388, C), BF16),
        ("bqc", (128, 2), F32),
        ("ohb", (2, 97), BF16),
        ("xpb", (C, N), F32),
    ]:
        io[name] = nc.dram_tensor(name, shape, dt_, kind="ExternalInput").ap()
    io["out"] = nc.dram_tensor("out", (C, N), F32, kind="ExternalOutput").ap()
    with tile.TileContext(nc) as tc:
        _emit(tc, io)
    nc.finalize()
    return nc


def host_prep(x, Wq, bq, Wk, bk, Wv, bv, Wo, bo):
    """Build per-core input maps (numpy only)."""
    bf16 = ml_dtypes.bfloat16
    x = np.ascontiguousarray(np.asarray(x, np.float32))
    Wq, bq = np.asarray(Wq, np.float32), np.asarray(bq, np.float32)
    Wk = np.asarray(Wk, np.float32)
    Wv, bv = np.asarray(Wv, np.float32), np.asarray(bv, np.float32)
    Wo, bo = np.asarray(Wo, np.float32), np.asarray(bo, np.float32)
    s = 1.0 / math.sqrt(HD)

    wq_hat = np.ascontiguousarray(Wq.T * s)          # [C, C] col 32h+d
    wk_hat = np.ascontiguousarray(Wk.T)
    bqc = (bq * s).reshape(2, 128).T                  # [128, 2] chunk cols
    wv_hat = np.ascontiguousarray(Wv.T)

    woT = Wo.T                                        # [C(d), C(out)]
    wop = np.zeros((388, C), np.float32)
    for p in range(4):
        wop[97 * p + 0 : 97 * p + 32, :] = woT[32 * (2 * p) : 32 * (2 * p) + 32, :]
        wop[97 * p + 64 : 97 * p + 96, :] = woT[32 * (2 * p + 1) : 32 * (2 * p + 1) + 32, :]

    ohb = np.zeros((2, 97), np.float32)
    ohb[0, 0:32] = 1.0
    ohb[1, 64:96] = 1.0

    bo2 = Wo @ bv + bo

    common = {
        "wq": wq_hat.astype(bf16),
        "wk": wk_hat.astype(bf16),
        "wv": wv_hat.astype(bf16),
        "wop": wop.astype(bf16),
        "bqc": np.ascontiguousarray(bqc),
        "ohb": ohb.astype(bf16),
    }

    B = x.shape[0]
    in_maps = []
    for b in range(B):
        xb = np.ascontiguousarray(x[b].reshape(C, N))
        m = dict(common)
        m["xb"] = xb.astype(bf16)
        m["xpb"] = np.ascontiguousarray(xb + bo2[:, None])
        in_maps.append(m)
    return in_maps


def kernel(x, Wq, bq, Wk, bk, Wv, bv, Wo, bo):
    global _NC, LAST_RESULTS
    if _NC is None:
        _NC = build_nc()
    in_maps = host_prep(x, Wq, bq, Wk, bk, Wv, bv, Wo, bo)
    res = run_bass_kernel_spmd(_NC, in_maps, core_ids=list(range(NCORES)))
    LAST_RESULTS = res
    out = np.stack([r["out"] for r in res.results], axis=0)
    return out.reshape(NCORES, C, 32, 32).astype(np.float32)


if __name__ == "__main__":
    rng = np.random.default_rng(0)
    ins = {
        "x": rng.standard_normal((8, C, 32, 32), dtype=np.float32),
        "Wq": rng.standard_normal((C, C), dtype=np.float32) / 16,
        "bq": rng.standard_normal(C).astype(np.float32) * 0.01,
        "Wk": rng.standard_normal((C, C), dtype=np.float32) / 16,
        "bk": rng.standard_normal(C).astype(np.float32) * 0.01,
        "Wv": rng.standard_normal((C, C), dtype=np.float32) / 16,
        "bv": rng.standard_normal(C).astype(np.float32) * 0.01,
        "Wo": rng.standard_normal((C, C), dtype=np.float32) / 16,
        "bo": rng.standard_normal(C).astype(np.float32) * 0.01,
    }
    out = kernel(**ins)
    # numpy reference
    x = ins["x"].reshape(8, C, N)
    q = np.einsum("oc,bcn->bon", ins["Wq"], x) + ins["bq"][None, :, None]
    k = np.einsum("oc,bcn->bon", ins["Wk"], x) + ins["bk"][None, :, None]
    v = np.einsum("oc,bcn->bon", ins["Wv"], x) + ins["bv"][None, :, None]
    q = q.reshape(8, NH, HD, N); k = k.reshape(8, NH, HD, N); v = v.reshape(8, NH, HD, N)
    sc = np.einsum("bhdn,bhdm->bhnm", q, k) / math.sqrt(HD)
    w = np.exp(sc - sc.max(-1, keepdims=True))
    w /= w.sum(-1, keepdims=True)
    o = np.einsum("bhnm,bhdm->bhnd", w, v)
    o = o.transpose(0, 1, 3, 2).reshape(8, C, N)
    exp = np.einsum("oc,bcn->bon", ins["Wo"], o) + ins["bo"][None, :, None] + x
    exp = exp.reshape(8, C, 32, 32)
    rel = np.linalg.norm(out - exp) / np.linalg.norm(exp)
    print("out", out.shape, "rel_err", rel)
